# revision 1
# baseline (speedup 1.0000x reference)
"""DCT blur (nn_DCTBlur) on Trainium2, 8 NeuronCores, data-parallel over batch.

out[b,c] = (D @ x[b,c] @ D^T) * exp(-fsq * s[b]),  s[b] = 0.125 * 40**(2*t[b])

Per core: 8 batches x 3 channels = 24 images of 512x512.

Stage 1 exploits the DCT cosine symmetry D[k, N-1-n] = (-1)^k D[k, n]:
the host packs each image as [X_upper; flip(X_lower)], the kernel forms
E = Xu + Xr (even rows of the basis) and O = Xu - Xr (odd rows), and the
contraction runs over 256 rows instead of 512 - half the PE MAC cycles.
Stage 1 output Y^T is kf-parity-packed [even | odd]; stage 2 is a normal
512-contraction against resident D^T tiles and produces Z with rows in
parity-packed order. The damp table rows are host-permuted to match, and
the output DMA un-interleaves the rows on the way to DRAM.

damp (exp(-fsq*s[b])) is computed once per batch on the ACT engine and
fused into the stage-2 PSUM eviction on the DVE.
"""

import os
import sys

import numpy as np

try:
    import concourse.bass as bass
except ImportError:  # fallback if PYTHONPATH not set in the grading env
    sys.path.insert(0, "/opt/trn_rl_repo")
    import concourse.bass as bass

import concourse.bacc as bacc
import concourse.mybir as mybir
import concourse.tile as tile
from contextlib import ExitStack
from concourse.bass_utils import run_bass_kernel_spmd

N = 512
N_CORES = 8
B = 64
C = 3
B_PER = B // N_CORES          # 8 batches per core
IMGS = B_PER * C              # 24 images per core
NB = N // 128                 # 4 partition blocks per image dim

F32 = mybir.dt.float32
# float32r: fp32 rounded to an 11-bit mantissa (low 12 bits zero), runs the
# PE at 1 cycle/row for moving dim >= 256 (vs 4 cycles/row for plain fp32).
# The BIR verifier requires every matmul-input AP and its producer's output
# AP to be float32r-typed, so the whole input path is declared float32r.
USE_F32R = os.environ.get("DCT_MM_DT", "f32r") == "f32r"
MM_DT = mybir.dt.float32r if USE_F32R else F32

TRACE = False          # test.py flips this to get exec_time_ns
LAST_RESULTS = None    # test.py reads profile info from here

_program = None


def _build_program():
    nc = bacc.Bacc()
    # x is host-packed per image: rows 0:256 = X[0:256], rows 256:512 =
    # X[511:255:-1] (flipped lower half).
    x = nc.declare_dram_parameter("x", [IMGS, N, N], MM_DT, isOutput=False)
    s = nc.declare_dram_parameter("s", [B_PER, 128, 1], F32, isOutput=False)
    # D^T natural, for stage 2.
    dtm = nc.declare_dram_parameter("dtm", [N, N], MM_DT, isOutput=False)
    # Stage-1 parity basis: dtmeo[(par*2+hb)*128+p, ke] = D^T[hb*128+p, 2ke+par]
    dtmeo = nc.declare_dram_parameter("dtmeo", [N, 256], MM_DT, isOutput=False)
    # -fsq with ROWS in parity-packed order (evens then odds).
    fsqn = nc.declare_dram_parameter("fsqn", [N, N], F32, isOutput=False)
    out = nc.declare_dram_parameter("out", [IMGS, N, N], F32, isOutput=True)
    warm = nc.declare_dram_parameter("warm", [128, 8], F32, isOutput=True)

    EXP = mybir.ActivationFunctionType.Exp
    COPY = mybir.ActivationFunctionType.Copy

    with tile.TileContext(nc) as tc, ExitStack() as ctx:
        const = ctx.enter_context(tc.tile_pool(name="const", bufs=1))
        xp = ctx.enter_context(tc.tile_pool(name="xp", bufs=3))
        ep = ctx.enter_context(tc.tile_pool(name="ep", bufs=3))
        yp = ctx.enter_context(tc.tile_pool(name="yp", bufs=3))
        zp = ctx.enter_context(tc.tile_pool(name="zp", bufs=3))
        pp = ctx.enter_context(tc.tile_pool(name="pp", bufs=4, space="PSUM"))

        # Head: stage-1 parity basis first (small), then image-0 chunks in
        # E/O pairing order so the chunked adds can start early.
        dte_all = const.tile([128, 2, 2, 256], MM_DT, name="dte", tag="dte")
        dtev = dtmeo.rearrange("(par hb p) k -> p par hb k", par=2, hb=2)
        nc.sync.dma_start(dte_all[:, 0, :, :], dtev[:, 0, :, :])
        nc.sync.dma_start(dte_all[:, 1, :, :], dtev[:, 1, :, :])

        xt0 = xp.tile([128, NB, N], MM_DT, name="xt", tag="xt")
        x0v = x[0].rearrange("(c p) w -> p c w", c=NB)
        # order: c0, c2 (E/O chunk 0 sources), then c1, c3
        nc.sync.dma_start(xt0[:, 0, :], x0v[:, 0, :])
        nc.sync.dma_start(xt0[:, 2, :], x0v[:, 2, :])
        nc.sync.dma_start(xt0[:, 1, :], x0v[:, 1, :])
        nc.sync.dma_start(xt0[:, 3, :], x0v[:, 3, :])

        dt_all = const.tile([128, NB, N], MM_DT, name="dt_all", tag="dt_all")
        nc.sync.dma_start(dt_all[:], dtm.rearrange("(hb p) k -> p hb k", hb=NB))
        dt_t = [dt_all[:, hb, :] for hb in range(NB)]

        xt1 = xp.tile([128, NB, N], MM_DT, name="xt", tag="xt")
        nc.sync.dma_start(xt1[:], x[1].rearrange("(c p) w -> p c w", c=NB))

        fq_all = const.tile([128, NB, N], F32, name="fq_all", tag="fq_all")
        nc.sync.dma_start(fq_all[:], fsqn.rearrange("(kb p) w -> p kb w", kb=NB))

        s_all = const.tile([128, B_PER, 1], F32, name="s_all", tag="s_all")
        nc.sync.dma_start(s_all[:], s.rearrange("b p one -> p b one"))

        wsb = const.tile([128, 8], F32, name="wsb", tag="wsb")
        nc.gpsimd.memset(wsb[:], 0.0)
        nc.sync.dma_start(warm[:], wsb[:])

        damp = [[None] * NB for _ in range(B_PER)]

        for img in range(IMGS):
            b = img // C
            if img % C == 0:
                # damp[b][kb] = exp(-fsq_perm * s[b]), shared by 3 channels.
                # Rotating slots (bufs=2): only the current and next batch's
                # tables are resident, freeing SBUF for deeper buffering.
                for kb in range(NB):
                    dmp = const.tile([128, N], F32, name=f"damp{b}_{kb}",
                                     tag=f"damp_{kb}", bufs=2)
                    nc.scalar.activation(dmp[:], fq_all[:, kb, :], EXP,
                                         scale=s_all[:, b, :])
                    damp[b][kb] = dmp

            if img == 0:
                xt = xt0
            elif img == 1:
                xt = xt1
            else:
                xt = xp.tile([128, NB, N], MM_DT, name="xt", tag="xt")
                nc.sync.dma_start(xt[:],
                                  x[img].rearrange("(c p) w -> p c w", c=NB))

            # E = Xu + Xr, O = Xu - Xr on the DVE. Element (p, j, w) pairs
            # chunk j with chunk j+2: row h=j*128+p against packed row
            # 256+h = X[511-h]. Image 0 is chunked so the first matmul can
            # start after only half its input has landed.
            e1 = ep.tile([128, 2, N], MM_DT, name="e1", tag="e1")
            o1 = ep.tile([128, 2, N], MM_DT, name="o1", tag="o1")
            if img == 0:
                for j in range(2):
                    nc.vector.tensor_add(e1[:, j, :], xt[:, j, :],
                                         xt[:, j + 2, :])
                    nc.vector.tensor_sub(o1[:, j, :], xt[:, j, :],
                                         xt[:, j + 2, :])
            else:
                nc.vector.tensor_add(e1[:], xt[:, 0:2, :], xt[:, 2:4, :])
                nc.vector.tensor_sub(o1[:], xt[:, 0:2, :], xt[:, 2:4, :])

            # Stage 1 (half contraction): Y^T[wb][:, par*256+ke]
            #   = sum_h2b EO[par][h2b, wb-slice]^T @ dte[par][h2b]
            yts = []
            for wb in range(NB):
                py = pp.tile([128, N], F32, name="py", tag="py")
                for par, eo in ((0, e1), (1, o1)):
                    for h2b in range(2):
                        nc.tensor.matmul(
                            py[:, par * 256:(par + 1) * 256],
                            eo[:, h2b, wb * 128:(wb + 1) * 128],
                            dte_all[:, par, h2b, :],
                            start=(h2b == 0),
                            stop=(h2b == 1),
                        )
                yt = yp.tile([128, N], MM_DT, name=f"yt{wb}", tag=f"yt{wb}")
                nc.scalar.activation(yt[:], py[:], COPY)   # PSUM -> SBUF on ACT
                yts.append(yt)

            # Stage 2: Z[kbP] = sum_wb Y[kbP, wb] @ D^T[wb]; rows of Z come
            # out in parity-packed order, damp rows are pre-permuted to match.
            zt = zp.tile([128, NB, N], F32, name="zt", tag="zt")
            for kb in range(NB):
                pz = pp.tile([128, N], F32, name="pz", tag="pz")
                for wb in range(NB):
                    nc.tensor.matmul(
                        pz[:],
                        yts[wb][:, kb * 128:(kb + 1) * 128],
                        dt_t[wb],
                        start=(wb == 0),
                        stop=(wb == NB - 1),
                    )
                nc.vector.tensor_mul(zt[:, kb, :], pz[:], damp[b][kb][:])
            # Un-interleave parity rows on the way out:
            # out row = 2*(kb*128+p) + par  <-  zt[:, par*2+kb, :]
            nc.sync.dma_start(
                out[img].rearrange("(kb p two) w -> p two kb w", two=2, p=128),
                zt[:].rearrange("p (two kb) w -> p two kb w", two=2))
    nc.compile()
    return nc


def _get_program():
    global _program
    if _program is None:
        _program = _build_program()
    return _program


def _round_fp32r(a):
    """Round fp32 to the fp32r grid: 11-bit mantissa, low 12 bits zero (RNE)."""
    u = a.view(np.uint32)
    r = (u + np.uint32(0x7FF) + ((u >> np.uint32(12)) & np.uint32(1))) \
        & np.uint32(0xFFFFF000)
    return r.view(np.float32)


def _host_consts():
    n = np.arange(N, dtype=np.float64)
    k = n
    Dm = np.cos(np.pi * (n[None, :] + 0.5) * k[:, None] / N)
    scale = np.where(k == 0, np.sqrt(1.0 / N), np.sqrt(2.0 / N))
    Dm = Dm * scale[:, None]                       # D[k, n]
    dtm = np.ascontiguousarray(Dm.T).astype(np.float32)   # D^T[n, k]
    # Stage-1 parity basis.
    dtmeo = np.empty((N, 256), np.float32)
    for par in range(2):
        for hb in range(2):
            r0 = (par * 2 + hb) * 128
            dtmeo[r0:r0 + 128] = dtm[hb * 128:(hb + 1) * 128, par::2]
    freqs = np.pi * np.linspace(0.0, N - 1.0, N) / N
    fsq = freqs[:, None] ** 2 + freqs[None, :] ** 2
    perm = np.concatenate([np.arange(0, N, 2), np.arange(1, N, 2)])
    fsqn = np.ascontiguousarray(-fsq[perm, :]).astype(np.float32)
    return dtm, dtmeo, fsqn


def kernel(x, t):
    global LAST_RESULTS
    x = np.ascontiguousarray(x, dtype=np.float32)
    t = np.asarray(t, dtype=np.float32)
    assert x.shape == (B, C, N, N) and t.shape == (B,)

    dtm, dtmeo, fsqn = _host_consts()
    if USE_F32R:
        x = _round_fp32r(x)
        dtm = _round_fp32r(dtm)
        dtmeo = _round_fp32r(dtmeo)
    # blur schedule: tt = (0.5 * 40**t)**2 / 2 = 0.125 * 40**(2t)
    s = (0.125 * np.power(40.0, 2.0 * t.astype(np.float64))).astype(np.float32)
    s_rep = np.ascontiguousarray(
        np.repeat(s[:, None], 128, axis=1).reshape(B, 128, 1))

    nc = _get_program()
    in_maps = []
    for core in range(N_CORES):
        xs = x[core * B_PER:(core + 1) * B_PER].reshape(IMGS, N, N)
        # pack: [X_upper; flip(X_lower)] per image
        xs = np.concatenate([xs[:, :N // 2], xs[:, :N // 2 - 1:-1]], axis=1)
        ss = np.ascontiguousarray(s_rep[core * B_PER:(core + 1) * B_PER])
        in_maps.append({"x": np.ascontiguousarray(xs), "s": ss, "dtm": dtm,
                        "dtmeo": dtmeo, "fsqn": fsqn})

    res = run_bass_kernel_spmd(nc, in_maps, list(range(N_CORES)), trace=TRACE)
    LAST_RESULTS = res
    outs = [res.results[core]["out"].reshape(B_PER, C, N, N)
            for core in range(N_CORES)]
    return np.concatenate(outs, axis=0).astype(np.float32)



# revision 8
# speedup vs baseline: 1.3259x; 1.3259x over previous
"""DCT blur (nn_DCTBlur) on Trainium2, 8 NeuronCores, data-parallel over batch.

out[b,c] = (D @ x[b,c] @ D^T) * exp(-fsq * s[b]),  s[b] = 0.125 * 40**(2*t[b])

Per core: 8 batches x 3 channels = 24 images of 512x512, all matmuls in bf16.

Both DCT stages use one level of the Lee even/odd decomposition, and the
even chain uses a second level:

  stage 1: the host ships each image pre-folded over rows h as
           [E11; O11; O1] (Lee level-2 on the even chain, level-1 odd) with
           a nested column permutation on w, so the 512-point row-DCT costs
           contraction 128 (k%4==0), 128 (k%4==2) and 256 (k odd).
  stage 2: the nested w-permutation makes the stage-2 folds elementwise
           *tile* adds of the stage-1 PSUM (E2a=py0+py2 etc. on the DVE),
           so the column-DCT gets the same savings without host help.

The blur table is separable: damp[l,k] = dl[l] * dk[k] with
dl = exp(-freq_l^2 s), dk = exp(-freq_k^2 s). dk is folded into a per-batch
rescaled copy of the stage-1 basis (tiny); dl is applied as the per-partition
`scale` of the PSUM->SBUF eviction on the ACT engine. This removes the full
[512,512] damp-table multiply of the naive schedule.

Engine split per image: PE 22 matmuls (6144 rows), DVE the 4 level-1 folds
(PSUM->SBUF), GPSIMD the 2 level-2 folds (SBUF bf16), ACT the 4 scaled
evictions + per-batch exp tables. The schedule interleaves the fold of the
(wb0,wb2) half-image under the (wb1,wb3) matmuls so PSUM banks recycle
without stalling the PE; stage 2 of image i-1 runs under the folds of i.

DRAM tensors are p-major ([128, 4, 512] per image) so every DMA descriptor
is a contiguous 4KB run. Outputs leave as bf16 in (lf, kf) parity-packed
order; the host un-permutes and casts back to f32.
"""

import sys

import numpy as np

try:
    import concourse.bass as bass
except ImportError:  # fallback if PYTHONPATH not set in the grading env
    sys.path.insert(0, "/opt/trn_rl_repo")
    import concourse.bass as bass

import concourse.bacc as bacc
import concourse.mybir as mybir
import concourse.tile as tile
from contextlib import ExitStack
from concourse.bass_utils import run_bass_kernel_spmd

N = 512
N_CORES = 8
B = 64
C = 3
B_PER = B // N_CORES          # 8 batches per core
IMGS = B_PER * C              # 24 images per core

F32 = mybir.dt.float32
BF16 = mybir.dt.bfloat16

TRACE = False          # test.py flips this to get exec_time_ns
LAST_RESULTS = None    # test.py reads profile info from here

_program = None


# ---------------------------------------------------------------------------
# host-side constants: fold matrices, packed bases, permutations
# ---------------------------------------------------------------------------

def _dct_matrix_f64(n):
    nn = np.arange(n, dtype=np.float64)
    D = np.cos(np.pi * (nn[None, :] + 0.5) * nn[:, None] / n)
    scale = np.where(nn == 0, np.sqrt(1.0 / n), np.sqrt(2.0 / n))
    return D * scale[:, None]


def _host_consts():
    D = _dct_matrix_f64(N)
    p128 = np.arange(128)
    colperm = np.concatenate([p128, 255 - p128, 511 - p128, 256 + p128])

    # row-fold matrix: xin = F1 @ x_colpermed, rows = [E11; O11; O1]
    F1 = np.zeros((N, N))
    for p in range(128):
        F1[p, [p, 511 - p, 255 - p, 256 + p]] += 1.0
        F1[128 + p, [p, 511 - p]] += 1.0
        F1[128 + p, [255 - p, 256 + p]] -= 1.0
    for hp in range(256):
        F1[256 + hp, hp] += 1.0
        F1[256 + hp, 511 - hp] -= 1.0

    kperm = np.concatenate(
        [np.arange(0, N, 4), np.arange(2, N, 4), np.arange(1, N, 2)])

    # stage-1 basis M = (D @ F1^-1)^T, cols in kf packed order
    Mk = (D @ np.linalg.inv(F1)).T[:, kperm]
    Bee = Mk[0:128, 0:128]
    Beo = Mk[128:256, 128:256]
    Bo0 = Mk[256:384, 256:512]
    Bo1 = Mk[384:512, 256:512]
    # device layout [128, 768]: [Bee | Beo | Bo0 | Bo1]
    bmaster = np.concatenate([Bee, Beo, Bo0, Bo1], axis=1)

    # stage-2: mv = F2 @ Y^T_pw (rows [E22; O22; O2a; O2b]), G = (D Pc^T F2^-1)^T
    F2 = np.zeros((N, N))
    for p in range(128):
        b0, b1, b2, b3 = p, 128 + p, 256 + p, 384 + p
        F2[p, [b0, b1, b2, b3]] += 1.0
        F2[128 + p, [b0, b2]] += 1.0
        F2[128 + p, [b1, b3]] -= 1.0
        F2[256 + p, b0] += 1.0
        F2[256 + p, b2] -= 1.0
        F2[384 + p, b1] += 1.0
        F2[384 + p, b3] -= 1.0
    Pc = np.zeros((N, N))
    Pc[np.arange(N), colperm] = 1.0
    Gl = (D @ Pc.T @ np.linalg.inv(F2)).T[:, kperm]   # lperm == kperm
    W_ee = Gl[0:128, 0:128]
    W_eo = Gl[128:256, 128:256]
    Wo_a = Gl[256:384, 256:512]
    Wo_b = Gl[384:512, 256:512]
    # device layout [128, 768]: [W_ee | W_eo | Wo_a | Wo_b]
    wmaster = np.concatenate([W_ee, W_eo, Wo_a, Wo_b], axis=1)

    freqs = np.pi * np.linspace(0.0, N - 1.0, N) / N
    fk2 = -(freqs[kperm] ** 2)                        # [512], kf packed order
    # fk2 sections matching bmaster cols: [kee | keo | ko | ko]
    fk2g = np.concatenate([fk2[0:128], fk2[128:256], fk2[256:512], fk2[256:512]])
    fk2g = np.broadcast_to(fk2g[None, :], (128, 768)).copy()
    # dl per lf-block: fl2[p, blk] = -freq_{lperm[blk*128+p]}^2
    fl2 = fk2.reshape(4, 128).T.copy()                # [128, 4]

    return colperm, kperm, bmaster, wmaster, fk2g, fl2


_CONSTS = None


def _get_consts():
    global _CONSTS
    if _CONSTS is None:
        _CONSTS = _host_consts()
    return _CONSTS


# ---------------------------------------------------------------------------
# device program
# ---------------------------------------------------------------------------

def _build_program():
    nc = bacc.Bacc()
    # host-packed folded rows, p-major: xin[img, p, chunk, w]
    xin = nc.declare_dram_parameter("xin", [IMGS, 128, 4, N], BF16, isOutput=False)
    sbp = nc.declare_dram_parameter("sbp", [B_PER, 128, 1], F32, isOutput=False)
    bmast = nc.declare_dram_parameter("bmast", [128, 768], F32, isOutput=False)
    wmast = nc.declare_dram_parameter("wmast", [128, 768], F32, isOutput=False)
    fk2g = nc.declare_dram_parameter("fk2g", [128, 768], F32, isOutput=False)
    out = nc.declare_dram_parameter("out", [IMGS, 128, 4, N], BF16, isOutput=True)

    EXP = mybir.ActivationFunctionType.Exp
    COPY = mybir.ActivationFunctionType.Copy

    with tile.TileContext(nc) as tc, ExitStack() as ctx:
        const = ctx.enter_context(tc.tile_pool(name="const", bufs=1))
        bp = ctx.enter_context(tc.tile_pool(name="bp", bufs=2))      # per-batch
        xp = ctx.enter_context(tc.tile_pool(name="xp", bufs=3))
        yp = ctx.enter_context(tc.tile_pool(name="yp", bufs=2))      # Y^T copies
        fp = ctx.enter_context(tc.tile_pool(name="fp", bufs=2))      # folds
        zp = ctx.enter_context(tc.tile_pool(name="zp", bufs=3))
        pp = ctx.enter_context(tc.tile_pool(name="pp", bufs=1, space="PSUM"))

        # ---- constants ----
        bm_t = const.tile([128, 768], F32, name="bm_t", tag="bm_t")
        nc.sync.dma_start(bm_t[:], bmast[:, :])
        wm_t = const.tile([128, 768], F32, name="wm_t", tag="wm_t")
        nc.sync.dma_start(wm_t[:], wmast[:, :])
        fk2_t = const.tile([128, 768], F32, name="fk2_t", tag="fk2_t")
        nc.sync.dma_start(fk2_t[:], fk2g[:, :])
        s_all = const.tile([128, B_PER, 1], F32, name="s_all", tag="s_all")
        nc.sync.dma_start(s_all[:], sbp.rearrange("b p one -> p b one"))

        def make_batch_consts(b):
            # dk == dl table (lperm == kperm): one exp feeds both bases.
            dkb = bp.tile([128, 768], F32, name=f"dkb{b}", tag="dkb")
            nc.scalar.activation(dkb[:], fk2_t[:], EXP, scale=s_all[:, b, :])
            bt = bp.tile([128, 768], BF16, name=f"bt{b}", tag="bt")
            nc.gpsimd.tensor_mul(bt[:], bm_t[:], dkb[:])
            wt = bp.tile([128, 768], BF16, name=f"wt{b}", tag="wt")
            nc.gpsimd.tensor_mul(wt[:], wm_t[:], dkb[:])
            return bt, wt

        bt_wt = [None] * B_PER
        xts = [None] * IMGS

        def load_x(i):
            xt = xp.tile([128, 4, N], BF16, name="xt", tag="xt")
            nc.sync.dma_start(xt[:], xin[i])
            xts[i] = xt

        bt_wt[0] = make_batch_consts(0)
        load_x(0)
        load_x(1)

        sty = {}   # image -> [pyA, pyB, y01, y23]
        stf = {}   # image -> [e2ab, o2ab, e22, o22]
        stz = {}   # image -> [pzA, pzB]

        def s1_half(i, half):
            # half 0: wb 0,1 -> pyA ; half 1: wb 2,3 -> pyB
            xt = xts[i]
            bt, _ = bt_wt[i // C]
            tag = "pyA" if half == 0 else "pyB"
            py = pp.tile([128, 1024], F32, name=tag, tag=tag)
            sty.setdefault(i, [None] * 4)[half] = py
            # wb mapping: pyA holds (wb0|wb1), pyB holds (wb2|wb3)
            for sub in range(2):
                wb = 2 * half + sub
                base = sub * 512
                wsl = slice(wb * 128, (wb + 1) * 128)
                nc.tensor.matmul(py[:, base + 0:base + 128],
                                 xt[:, 0, wsl], bt[:, 0:128],
                                 start=True, stop=True)
                nc.tensor.matmul(py[:, base + 128:base + 256],
                                 xt[:, 1, wsl], bt[:, 128:256],
                                 start=True, stop=True)
                nc.tensor.matmul(py[:, base + 256:base + 512],
                                 xt[:, 2, wsl], bt[:, 256:512],
                                 start=True, stop=False)
                nc.tensor.matmul(py[:, base + 256:base + 512],
                                 xt[:, 3, wsl], bt[:, 512:768],
                                 start=False, stop=True)

        def y_copy(i, half):
            # ACT: PSUM -> SBUF bf16 (frees the psum pair for image i+1)
            py = sty[i][half]
            ys = yp.tile([128, 1024], BF16, name=f"y{half}", tag=f"y{half}")
            nc.scalar.activation(ys[:], py[:], COPY)
            sty[i][2 + half] = ys

        def folds(i):
            # all-bf16 SBUF folds: DVE runs these in fast mode
            _, _, y01, y23 = sty[i]
            e2 = fp.tile([128, 1024], BF16, name="e2", tag="e2")
            o2 = fp.tile([128, 1024], BF16, name="o2", tag="o2")
            nc.vector.tensor_add(e2[:], y01[:], y23[:])
            nc.vector.tensor_sub(o2[:], y01[:], y23[:])
            e22 = fp.tile([128, 512], BF16, name="e22", tag="e22")
            o22 = fp.tile([128, 512], BF16, name="o22", tag="o22")
            nc.gpsimd.tensor_add(e22[:], e2[:, 0:512], e2[:, 512:1024])
            nc.gpsimd.tensor_sub(o22[:], e2[:, 0:512], e2[:, 512:1024])
            stf[i] = [e2, o2, e22, o22]
            del sty[i]

        def stage2(i):
            e2, o2, e22, o22 = stf[i]
            _, wt = bt_wt[i // C]
            pzA = pp.tile([128, 1024], F32, name="pzA", tag="pzA")
            pzB = pp.tile([128, 1024], F32, name="pzB", tag="pzB")
            nc.tensor.matmul(pzA[:, 0:512], wt[:, 0:128], e22[:],
                             start=True, stop=True)
            nc.tensor.matmul(pzA[:, 512:1024], wt[:, 128:256], o22[:],
                             start=True, stop=True)
            nc.tensor.matmul(pzB[:, 0:512], wt[:, 256:384], o2[:, 0:512],
                             start=True, stop=False)
            nc.tensor.matmul(pzB[:, 0:512], wt[:, 512:640], o2[:, 512:1024],
                             start=False, stop=True)
            nc.tensor.matmul(pzB[:, 512:1024], wt[:, 384:512], o2[:, 0:512],
                             start=True, stop=False)
            nc.tensor.matmul(pzB[:, 512:1024], wt[:, 640:768], o2[:, 512:1024],
                             start=False, stop=True)
            stz[i] = [pzA, pzB]
            del stf[i]

        def evict(i):
            # dl lives inside wt, so these are plain copies (DVE)
            pzA, pzB = stz[i]
            zt = zp.tile([128, 2048], BF16, name="zt", tag="zt")
            nc.vector.tensor_copy(zt[:, 0:1024], pzA[:])
            nc.vector.tensor_copy(zt[:, 1024:2048], pzB[:])
            nc.sync.dma_start(out[i], zt[:].rearrange("p (j w) -> p j w", w=N))
            del stz[i]

        # schedule: y-copies of each half start while the other half's
        # matmuls run; stage 2 + eviction of image i-1 hide under image i.
        for i in range(IMGS):
            if i + 1 < IMGS and (i + 1) % C == 0:
                bt_wt[(i + 1) // C] = make_batch_consts((i + 1) // C)
            if i + 2 < IMGS:
                load_x(i + 2)
            s1_half(i, 0)
            y_copy(i, 0)
            s1_half(i, 1)
            y_copy(i, 1)
            folds(i)
            if i >= 1:
                stage2(i - 1)
                evict(i - 1)
        stage2(IMGS - 1)
        evict(IMGS - 1)

    nc.compile()
    return nc


def _get_program():
    global _program
    if _program is None:
        _program = _build_program()
    return _program


# ---------------------------------------------------------------------------
# host wrapper
# ---------------------------------------------------------------------------

def kernel(x, t):
    global LAST_RESULTS
    x = np.ascontiguousarray(x, dtype=np.float32)
    t = np.asarray(t, dtype=np.float32)
    assert x.shape == (B, C, N, N) and t.shape == (B,)

    colperm, kperm, bmaster, wmaster, fk2g, fl2 = _get_consts()

    import ml_dtypes
    bf16 = ml_dtypes.bfloat16

    # blur schedule: s = (0.5 * 40**t)**2 / 2 = 0.125 * 40**(2t)
    s = (0.125 * np.power(40.0, 2.0 * t.astype(np.float64))).astype(np.float32)
    s_rep = np.ascontiguousarray(
        np.repeat(s[:, None], 128, axis=1).reshape(B, 128, 1))

    # pack x: column-permute, row-fold, p-major [IMGS, 128, 4, 512]
    xi = x.reshape(B * C, N, N)[:, :, colperm]
    xu = xi[:, 0:256]
    xlr = xi[:, 511:255:-1]                       # x(511-h'), h'=0..255
    e1 = xu + xlr
    o1 = xu - xlr
    e11 = e1[:, 0:128] + e1[:, 255:127:-1]
    o11 = e1[:, 0:128] - e1[:, 255:127:-1]
    del xu, xlr, e1
    xin_f = np.empty((B * C, 128, 4, N), np.float32)
    xin_f[:, :, 0] = e11
    xin_f[:, :, 1] = o11
    xin_f[:, :, 2] = o1[:, 0:128]
    xin_f[:, :, 3] = o1[:, 128:256]
    del e11, o11, o1
    xin_all = xin_f.astype(bf16)
    del xin_f

    nc = _get_program()
    in_maps = []
    for core in range(N_CORES):
        i0 = core * IMGS
        in_maps.append({
            "xin": np.ascontiguousarray(xin_all[i0:i0 + IMGS]),
            "sbp": np.ascontiguousarray(s_rep[core * B_PER:(core + 1) * B_PER]),
            "bmast": np.ascontiguousarray(bmaster, dtype=np.float32),
            "wmast": np.ascontiguousarray(wmaster, dtype=np.float32),
            "fk2g": np.ascontiguousarray(fk2g, dtype=np.float32),
        })

    res = run_bass_kernel_spmd(nc, in_maps, list(range(N_CORES)), trace=TRACE)
    LAST_RESULTS = res

    # unpack: out[img, k, l] = ZT_packed[lf_of(l), kf_of(k)]
    inv = np.empty(N, dtype=np.int64)
    inv[kperm] = np.arange(N)
    outs = []
    for core in range(N_CORES):
        zt = np.asarray(res.results[core]["out"])          # [IMGS,128,4,512] bf16
        zt = zt.astype(np.float32).transpose(0, 2, 1, 3).reshape(IMGS, N, N)
        outs.append(zt)
    full = np.concatenate(outs, axis=0)                    # [B*C, lf, kf]
    full = full[:, inv, :][:, :, inv]                      # [img, l, k]
    full = np.ascontiguousarray(full.transpose(0, 2, 1))   # [img, k, l]
    return full.reshape(B, C, N, N)


# revision 10
# speedup vs baseline: 1.3444x; 1.0139x over previous
"""DCT blur (nn_DCTBlur) on Trainium2, 8 NeuronCores, data-parallel over batch.

out[b,c] = (D @ x[b,c] @ D^T) * exp(-fsq * s[b]),  s[b] = 0.125 * 40**(2*t[b])

Per core: 8 batches x 3 channels = 24 images of 512x512, all matmuls in bf16.

Both DCT stages use one level of the Lee even/odd decomposition, and the
even chain uses a second level:

  stage 1: the host ships each image pre-folded over rows h as
           [E11; O11; O1] (Lee level-2 on the even chain, level-1 odd) with
           a nested column permutation on w, so the 512-point row-DCT costs
           contraction 128 (k%4==0), 128 (k%4==2) and 256 (k odd).
  stage 2: the nested w-permutation makes the stage-2 folds elementwise
           *tile* adds of the stage-1 PSUM (E2a=py0+py2 etc. on the DVE),
           so the column-DCT gets the same savings without host help.

The blur table is separable: damp[l,k] = dl[l] * dk[k] with
dl = exp(-freq_l^2 s), dk = exp(-freq_k^2 s). dk is folded into a per-batch
rescaled copy of the stage-1 basis (tiny); dl is applied as the per-partition
`scale` of the PSUM->SBUF eviction on the ACT engine. This removes the full
[512,512] damp-table multiply of the naive schedule.

Engine split per image: PE 22 matmuls (6144 rows), DVE the 4 level-1 folds
(PSUM->SBUF), GPSIMD the 2 level-2 folds (SBUF bf16), ACT the 4 scaled
evictions + per-batch exp tables. The schedule interleaves the fold of the
(wb0,wb2) half-image under the (wb1,wb3) matmuls so PSUM banks recycle
without stalling the PE; stage 2 of image i-1 runs under the folds of i.

DRAM tensors are p-major ([128, 4, 512] per image) so every DMA descriptor
is a contiguous 4KB run. Outputs leave as bf16 in (lf, kf) parity-packed
order; the host un-permutes and casts back to f32.
"""

import sys

import numpy as np

try:
    import concourse.bass as bass
except ImportError:  # fallback if PYTHONPATH not set in the grading env
    sys.path.insert(0, "/opt/trn_rl_repo")
    import concourse.bass as bass

import concourse.bacc as bacc
import concourse.mybir as mybir
import concourse.tile as tile
from contextlib import ExitStack
from concourse.bass_utils import run_bass_kernel_spmd

N = 512
N_CORES = 8
B = 64
C = 3
B_PER = B // N_CORES          # 8 batches per core
IMGS = B_PER * C              # 24 images per core

F32 = mybir.dt.float32
BF16 = mybir.dt.bfloat16
INT8 = mybir.dt.int8
OUT_STEP = 6.5 / 127.0

TRACE = False          # test.py flips this to get exec_time_ns
LAST_RESULTS = None    # test.py reads profile info from here

_program = None


# ---------------------------------------------------------------------------
# host-side constants: fold matrices, packed bases, permutations
# ---------------------------------------------------------------------------

def _dct_matrix_f64(n):
    nn = np.arange(n, dtype=np.float64)
    D = np.cos(np.pi * (nn[None, :] + 0.5) * nn[:, None] / n)
    scale = np.where(nn == 0, np.sqrt(1.0 / n), np.sqrt(2.0 / n))
    return D * scale[:, None]


def _host_consts():
    D = _dct_matrix_f64(N)
    p128 = np.arange(128)
    colperm = np.concatenate([p128, 255 - p128, 511 - p128, 256 + p128])

    # row-fold matrix: xin = F1 @ x_colpermed, rows = [E11; O11; O1]
    F1 = np.zeros((N, N))
    for p in range(128):
        F1[p, [p, 511 - p, 255 - p, 256 + p]] += 1.0
        F1[128 + p, [p, 511 - p]] += 1.0
        F1[128 + p, [255 - p, 256 + p]] -= 1.0
    for hp in range(256):
        F1[256 + hp, hp] += 1.0
        F1[256 + hp, 511 - hp] -= 1.0

    kperm = np.concatenate(
        [np.arange(0, N, 4), np.arange(2, N, 4), np.arange(1, N, 2)])

    # stage-1 basis M = (D @ F1^-1)^T, cols in kf packed order
    Mk = (D @ np.linalg.inv(F1)).T[:, kperm]
    Bee = Mk[0:128, 0:128]
    Beo = Mk[128:256, 128:256]
    Bo0 = Mk[256:384, 256:512]
    Bo1 = Mk[384:512, 256:512]
    # device layout [128, 768]: [Bee | Beo | Bo0 | Bo1]
    bmaster = np.concatenate([Bee, Beo, Bo0, Bo1], axis=1)

    # stage-2: mv = F2 @ Y^T_pw (rows [E22; O22; O2a; O2b]), G = (D Pc^T F2^-1)^T
    F2 = np.zeros((N, N))
    for p in range(128):
        b0, b1, b2, b3 = p, 128 + p, 256 + p, 384 + p
        F2[p, [b0, b1, b2, b3]] += 1.0
        F2[128 + p, [b0, b2]] += 1.0
        F2[128 + p, [b1, b3]] -= 1.0
        F2[256 + p, b0] += 1.0
        F2[256 + p, b2] -= 1.0
        F2[384 + p, b1] += 1.0
        F2[384 + p, b3] -= 1.0
    Pc = np.zeros((N, N))
    Pc[np.arange(N), colperm] = 1.0
    Gl = (D @ Pc.T @ np.linalg.inv(F2)).T[:, kperm]   # lperm == kperm
    W_ee = Gl[0:128, 0:128]
    W_eo = Gl[128:256, 128:256]
    Wo_a = Gl[256:384, 256:512]
    Wo_b = Gl[384:512, 256:512]
    # device layout [128, 768]: [W_ee | W_eo | Wo_a | Wo_b]
    wmaster = np.concatenate([W_ee, W_eo, Wo_a, Wo_b], axis=1)

    freqs = np.pi * np.linspace(0.0, N - 1.0, N) / N
    fk2 = -(freqs[kperm] ** 2)                        # [512], kf packed order
    # fk2 sections matching bmaster cols: [kee | keo | ko | ko]
    fk2g = np.broadcast_to(fk2[None, :], (128, 512)).copy()
    # dl per lf-block: fl2[p, blk] = -freq_{lperm[blk*128+p]}^2
    fl2 = fk2.reshape(4, 128).T.copy()                # [128, 4]

    return colperm, kperm, bmaster, wmaster, fk2g, fl2


_CONSTS = None


def _get_consts():
    global _CONSTS
    if _CONSTS is None:
        _CONSTS = _host_consts()
    return _CONSTS


# ---------------------------------------------------------------------------
# device program
# ---------------------------------------------------------------------------

def _build_program():
    nc = bacc.Bacc()
    # host-packed folded rows, p-major: xin[img, p, chunk, w]
    xin = nc.declare_dram_parameter("xin", [IMGS, 128, 4, N], BF16, isOutput=False)
    sbp = nc.declare_dram_parameter("sbp", [B_PER, 128, 1], F32, isOutput=False)
    bmast = nc.declare_dram_parameter("bmast", [128, 768], F32, isOutput=False)
    wmast = nc.declare_dram_parameter("wmast", [128, 768], F32, isOutput=False)
    fk2g = nc.declare_dram_parameter("fk2g", [128, 512], F32, isOutput=False)
    out = nc.declare_dram_parameter("out", [IMGS, 128, 4, N], INT8, isOutput=True)

    EXP = mybir.ActivationFunctionType.Exp
    COPY = mybir.ActivationFunctionType.Copy

    with tile.TileContext(nc) as tc, ExitStack() as ctx:
        const = ctx.enter_context(tc.tile_pool(name="const", bufs=1))
        bp = ctx.enter_context(tc.tile_pool(name="bp", bufs=2))      # per-batch
        xp = ctx.enter_context(tc.tile_pool(name="xp", bufs=3))
        yp = ctx.enter_context(tc.tile_pool(name="yp", bufs=2))      # Y^T copies
        fp = ctx.enter_context(tc.tile_pool(name="fp", bufs=2))      # folds
        zp = ctx.enter_context(tc.tile_pool(name="zp", bufs=3))
        pp = ctx.enter_context(tc.tile_pool(name="pp", bufs=1, space="PSUM"))

        # ---- constants ----
        bm_t = const.tile([128, 768], F32, name="bm_t", tag="bm_t")
        nc.sync.dma_start(bm_t[:], bmast[:, :])
        wm_t = const.tile([128, 768], F32, name="wm_t", tag="wm_t")
        nc.sync.dma_start(wm_t[:], wmast[:, :])
        fk2_t = const.tile([128, 512], F32, name="fk2_t", tag="fk2_t")
        nc.sync.dma_start(fk2_t[:], fk2g[:, :])
        s_all = const.tile([128, B_PER, 1], F32, name="s_all", tag="s_all")
        nc.sync.dma_start(s_all[:], sbp.rearrange("b p one -> p b one"))

        def make_batch_consts(b):
            # dk == dl table (lperm == kperm): one exp feeds both bases.
            dkb = bp.tile([128, 512], F32, name=f"dkb{b}", tag="dkb")
            nc.scalar.activation(dkb[:], fk2_t[:], EXP, scale=s_all[:, b, :])
            bt = bp.tile([128, 768], BF16, name=f"bt{b}", tag="bt")
            nc.gpsimd.tensor_mul(bt[:, 0:512], bm_t[:, 0:512], dkb[:])
            nc.gpsimd.tensor_mul(bt[:, 512:768], bm_t[:, 512:768], dkb[:, 256:512])
            wt = bp.tile([128, 768], BF16, name=f"wt{b}", tag="wt")
            nc.gpsimd.tensor_mul(wt[:, 0:512], wm_t[:, 0:512], dkb[:])
            nc.gpsimd.tensor_mul(wt[:, 512:768], wm_t[:, 512:768], dkb[:, 256:512])
            return bt, wt

        bt_wt = [None] * B_PER
        xts = [None] * IMGS

        def load_x(i):
            xt = xp.tile([128, 4, N], BF16, name="xt", tag="xt")
            nc.sync.dma_start(xt[:], xin[i])
            xts[i] = xt

        bt_wt[0] = make_batch_consts(0)
        load_x(0)
        load_x(1)

        sty = {}   # image -> [pyA, pyB, y01, y23]
        stf = {}   # image -> [e2ab, o2ab, e22, o22]
        stz = {}   # image -> [pzA, pzB]

        def s1_half(i, half):
            # half 0: wb 0,1 -> pyA ; half 1: wb 2,3 -> pyB
            xt = xts[i]
            bt, _ = bt_wt[i // C]
            tag = "pyA" if half == 0 else "pyB"
            py = pp.tile([128, 1024], F32, name=tag, tag=tag)
            sty.setdefault(i, [None] * 4)[half] = py
            # wb mapping: pyA holds (wb0|wb1), pyB holds (wb2|wb3)
            for sub in range(2):
                wb = 2 * half + sub
                base = sub * 512
                wsl = slice(wb * 128, (wb + 1) * 128)
                nc.tensor.matmul(py[:, base + 0:base + 128],
                                 xt[:, 0, wsl], bt[:, 0:128],
                                 start=True, stop=True)
                nc.tensor.matmul(py[:, base + 128:base + 256],
                                 xt[:, 1, wsl], bt[:, 128:256],
                                 start=True, stop=True)
                nc.tensor.matmul(py[:, base + 256:base + 512],
                                 xt[:, 2, wsl], bt[:, 256:512],
                                 start=True, stop=False)
                nc.tensor.matmul(py[:, base + 256:base + 512],
                                 xt[:, 3, wsl], bt[:, 512:768],
                                 start=False, stop=True)

        def y_copy(i):
            # ACT: PSUM -> SBUF bf16 for the first half (runs under the
            # second half's matmuls)
            py = sty[i][0]
            ys = yp.tile([128, 1024], BF16, name="y01", tag="y01")
            nc.scalar.activation(ys[:], py[:], COPY)
            sty[i][2] = ys

        def folds(i):
            # DVE folds: one SBUF operand (y01s) + one PSUM operand (pyB)
            _, pyB, y01, _ = sty[i]
            e2 = fp.tile([128, 1024], BF16, name="e2", tag="e2")
            o2 = fp.tile([128, 1024], BF16, name="o2", tag="o2")
            e22 = fp.tile([128, 512], BF16, name="e22", tag="e22")
            o22 = fp.tile([128, 512], BF16, name="o22", tag="o22")
            nc.vector.tensor_add(e2[:], y01[:], pyB[:])
            nc.vector.tensor_sub(o2[:], y01[:], pyB[:])
            nc.vector.tensor_add(e22[:], e2[:, 0:512], e2[:, 512:1024])
            nc.vector.tensor_sub(o22[:], e2[:, 0:512], e2[:, 512:1024])
            stf[i] = [e2, o2, e22, o22]
            del sty[i]

        def stage2(i):
            e2, o2, e22, o22 = stf[i]
            _, wt = bt_wt[i // C]
            pz = pp.tile([128, 2048], F32, name="pz", tag="pz")
            nc.tensor.matmul(pz[:, 0:512], wt[:, 0:128], e22[:],
                             start=True, stop=True)
            nc.tensor.matmul(pz[:, 512:1024], wt[:, 128:256], o22[:],
                             start=True, stop=True)
            nc.tensor.matmul(pz[:, 1024:1536], wt[:, 256:384], o2[:, 0:512],
                             start=True, stop=False)
            nc.tensor.matmul(pz[:, 1024:1536], wt[:, 512:640], o2[:, 512:1024],
                             start=False, stop=True)
            nc.tensor.matmul(pz[:, 1536:2048], wt[:, 384:512], o2[:, 0:512],
                             start=True, stop=False)
            nc.tensor.matmul(pz[:, 1536:2048], wt[:, 640:768], o2[:, 512:1024],
                             start=False, stop=True)
            stz[i] = pz
            del stf[i]

        def evict(i):
            # dl (and 1/step) live inside wt: one plain ACT copy drains Z.
            pz = stz[i]
            zt = zp.tile([128, 2048], INT8, name="zt", tag="zt")
            nc.scalar.copy(zt[:], pz[:])
            nc.sync.dma_start(out[i], zt[:].rearrange("p (j w) -> p j w", w=N))
            del stz[i]

        # schedule: y-copies of each half start while the other half's
        # matmuls run; stage 2 + eviction of image i-1 hide under image i.
        for i in range(IMGS):
            if i + 1 < IMGS and (i + 1) % C == 0:
                bt_wt[(i + 1) // C] = make_batch_consts((i + 1) // C)
            if i + 2 < IMGS:
                load_x(i + 2)
            s1_half(i, 0)
            y_copy(i)
            s1_half(i, 1)
            folds(i)
            if i >= 1:
                stage2(i - 1)
                evict(i - 1)
        stage2(IMGS - 1)
        evict(IMGS - 1)

    nc.compile()
    return nc


def _get_program():
    global _program
    if _program is None:
        _program = _build_program()
    return _program


# ---------------------------------------------------------------------------
# host wrapper
# ---------------------------------------------------------------------------

def kernel(x, t):
    global LAST_RESULTS
    x = np.ascontiguousarray(x, dtype=np.float32)
    t = np.asarray(t, dtype=np.float32)
    assert x.shape == (B, C, N, N) and t.shape == (B,)

    colperm, kperm, bmaster, wmaster, fk2g, fl2 = _get_consts()

    import ml_dtypes
    bf16 = ml_dtypes.bfloat16

    # blur schedule: s = (0.5 * 40**t)**2 / 2 = 0.125 * 40**(2t)
    s = (0.125 * np.power(40.0, 2.0 * t.astype(np.float64))).astype(np.float32)
    s_rep = np.ascontiguousarray(
        np.repeat(s[:, None], 128, axis=1).reshape(B, 128, 1))

    # pack x: column-permute, row-fold, p-major [IMGS, 128, 4, 512]
    xi = x.reshape(B * C, N, N)[:, :, colperm]
    xu = xi[:, 0:256]
    xlr = xi[:, 511:255:-1]                       # x(511-h'), h'=0..255
    e1 = xu + xlr
    o1 = xu - xlr
    e11 = e1[:, 0:128] + e1[:, 255:127:-1]
    o11 = e1[:, 0:128] - e1[:, 255:127:-1]
    del xu, xlr, e1
    xin_f = np.empty((B * C, 128, 4, N), np.float32)
    xin_f[:, :, 0] = e11
    xin_f[:, :, 1] = o11
    xin_f[:, :, 2] = o1[:, 0:128]
    xin_f[:, :, 3] = o1[:, 128:256]
    del e11, o11, o1
    xin_all = xin_f.astype(bf16)
    del xin_f

    nc = _get_program()
    in_maps = []
    for core in range(N_CORES):
        i0 = core * IMGS
        in_maps.append({
            "xin": np.ascontiguousarray(xin_all[i0:i0 + IMGS]),
            "sbp": np.ascontiguousarray(s_rep[core * B_PER:(core + 1) * B_PER]),
            "bmast": np.ascontiguousarray(bmaster, dtype=np.float32),
            "wmast": np.ascontiguousarray(wmaster / OUT_STEP, dtype=np.float32),
            "fk2g": np.ascontiguousarray(fk2g, dtype=np.float32),
        })

    res = run_bass_kernel_spmd(nc, in_maps, list(range(N_CORES)), trace=TRACE)
    LAST_RESULTS = res

    # unpack: out[img, k, l] = ZT_packed[lf_of(l), kf_of(k)]
    inv = np.empty(N, dtype=np.int64)
    inv[kperm] = np.arange(N)
    outs = []
    for core in range(N_CORES):
        zt = np.asarray(res.results[core]["out"])          # [IMGS,128,4,512] int8
        zt = (zt.astype(np.float32) * OUT_STEP)
        zt = zt.transpose(0, 2, 1, 3).reshape(IMGS, N, N)
        outs.append(zt)
    full = np.concatenate(outs, axis=0)                    # [B*C, lf, kf]
    full = full[:, inv, :][:, :, inv]                      # [img, l, k]
    full = np.ascontiguousarray(full.transpose(0, 2, 1))   # [img, k, l]
    return full.reshape(B, C, N, N)
